# revision 7
# baseline (speedup 1.0000x reference)
"""AttentionSE3 message-passing kernel for 8 Trainium2 NeuronCores.

Strategy (node-sharded, zero device collectives):
  - The softmax over incoming edges of each dst node is computed WITHOUT the
    max-subtraction (prelogits are ~N(0, 0.35^2) so exp never overflows), and
    the division by the softmax denominator commutes with the segment-sum:
        feat[n] = (sum_{e->n} exp(pl_e) * v_e) / (sum_{e->n} exp(pl_e))
    so each core only needs the edges whose dst it owns.
  - Host: sort edges by dst, route each edge to the core owning its dst node
    (2500 nodes/core), group into 20 node-tiles of 128 nodes, pad each
    node-tile's edge list to ET*128 slots. Gather key/value/q[dst] rows into
    the padded layout.
  - Device (per core): for each node-tile, build a 0/1 one-hot matrix
    oh[e, n] = (rel[e] == n) on the VectorE, compute per-edge/per-head logits
    as an elementwise mul + grouped reduce, exp on ScalarE, then segment-sum
    s and exp*value with a single TensorE matmul  feat = oh^T @ [exv | ex].
  - Host: un-permute prelogits, divide feat by s, slice outputs.
"""

import math
import os

import numpy as np

# ---- problem constants (hardcoded; must match reference.setup_inputs) ----
N = 20000
E = 640000
H = 8
DHEAD = 16
F = 128            # NUM_FEATURES = H * DHEAD
CV = 192           # value channels flattened (64*3), layout [8h, 8c, 3d]
CD = CV + H        # 200: value channels + per-head ex column for s
NCORES = 8
NPC = N // NCORES  # 2500 nodes per core
TILE = 128
NT = 20            # node tiles per core  (20*128 = 2560 >= 2500)
ET = 34            # max edge tiles per node tile (max count 4252 <= 4352)
SLOTS = ET * TILE
ETC = 17           # edge tiles per chunk
NCH = ET // ETC    # 2
SCALE = 1.0 / math.sqrt(float(F))

_COMPILED = None
LAST_RESULTS = None


def _build_nc(nt_count=NT, et=ET, etc=ETC, compile=True):
    import concourse.bass as bass
    import concourse.tile as tile
    from concourse import bacc, mybir

    nch = et // etc
    assert nch * etc == et

    fp32 = mybir.dt.float32
    nc = bacc.Bacc(
        "TRN2",
        target_bir_lowering=False,
        debug=False,
        enable_asserts=False,
        num_devices=NCORES,
    )

    key_d = nc.dram_tensor("key_r", [nt_count, TILE, et, F], fp32, kind="ExternalInput").ap()
    qg_d = nc.dram_tensor("qg_r", [nt_count, TILE, et, F], fp32, kind="ExternalInput").ap()
    val_d = nc.dram_tensor("val_r", [nt_count, TILE, et, CV], fp32, kind="ExternalInput").ap()
    rel_d = nc.dram_tensor("rel_r", [nt_count, TILE, et], fp32, kind="ExternalInput").ap()
    feat_d = nc.dram_tensor("feat_r", [nt_count, TILE, CD], fp32, kind="ExternalOutput").ap()
    prelog_d = nc.dram_tensor(
        "prelog_r", [nt_count, TILE, et, H], fp32, kind="ExternalOutput"
    ).ap()

    with tile.TileContext(nc) as tc:
        with (
            tc.tile_pool(name="consts", bufs=1) as consts,
            tc.tile_pool(name="io", bufs=3) as io,
            tc.tile_pool(name="work", bufs=3) as work,
            tc.tile_pool(name="outp", bufs=2) as outp,
            tc.tile_pool(name="pfeat", bufs=2, space="PSUM") as pfeat,
        ):
            iota_t = consts.tile([TILE, TILE], fp32)
            nc.gpsimd.iota(
                iota_t[:],
                [[1, TILE]],
                channel_multiplier=0,
                allow_small_or_imprecise_dtypes=True,
            )

            for nt in range(nt_count):
                feat_p = pfeat.tile([TILE, CD], fp32)
                for ch in range(nch):
                    et0 = ch * etc
                    k_t = io.tile([TILE, etc, F], fp32, tag="k")
                    nc.sync.dma_start(k_t[:], key_d[nt, :, et0 : et0 + etc, :])
                    qg_t = io.tile([TILE, etc, F], fp32, tag="qg")
                    nc.sync.dma_start(qg_t[:], qg_d[nt, :, et0 : et0 + etc, :])
                    v_t = io.tile([TILE, etc, CV], fp32, tag="v")
                    nc.sync.dma_start(v_t[:], val_d[nt, :, et0 : et0 + etc, :])
                    rel_t = io.tile([TILE, etc], fp32, tag="rel")
                    nc.sync.dma_start(rel_t[:], rel_d[nt, :, et0 : et0 + etc])

                    # one-hot: oh[p, et, j] = (rel[p, et] == j)
                    oh_t = work.tile([TILE, etc, TILE], fp32, tag="oh")
                    nc.vector.tensor_tensor(
                        out=oh_t[:],
                        in0=iota_t[:].unsqueeze(1).to_broadcast([TILE, etc, TILE]),
                        in1=rel_t[:].unsqueeze(2).to_broadcast([TILE, etc, TILE]),
                        op=mybir.AluOpType.is_equal,
                    )

                    # kq = k * q_gathered (in place over k)
                    nc.vector.tensor_tensor(
                        out=k_t[:], in0=k_t[:], in1=qg_t[:], op=mybir.AluOpType.mult
                    )
                    # logits[p, et, h] = sum_d kq[p, et, h, d]
                    logit_t = work.tile([TILE, etc, H], fp32, tag="logit")
                    nc.vector.tensor_reduce(
                        out=logit_t[:],
                        in_=k_t[:].rearrange("p e (h d) -> p e h d", d=DHEAD),
                        axis=mybir.AxisListType.X,
                        op=mybir.AluOpType.add,
                    )

                    # prelogits output = logits * SCALE
                    prelog_t = outp.tile([TILE, etc, H], fp32, tag="prelog")
                    nc.scalar.activation(
                        prelog_t[:],
                        logit_t[:],
                        mybir.ActivationFunctionType.Copy,
                        scale=SCALE,
                    )
                    nc.sync.dma_start(
                        prelog_d[nt, :, et0 : et0 + etc, :], prelog_t[:]
                    )

                    # exv[:, :, 192:200] = exp(logits * SCALE)
                    exv_t = work.tile([TILE, etc, CD], fp32, tag="exv")
                    nc.scalar.activation(
                        exv_t[:, :, CV:CD],
                        logit_t[:],
                        mybir.ActivationFunctionType.Exp,
                        scale=SCALE,
                    )
                    # exv[:, :, 0:192] = v * ex (broadcast ex over 24 channels)
                    # on GPSIMD: DVE is the critical engine, GpSimd is idle
                    nc.gpsimd.tensor_tensor(
                        out=exv_t[:, :, 0:CV].rearrange("p e (h c) -> p e h c", c=24),
                        in0=v_t[:].rearrange("p e (h c) -> p e h c", c=24),
                        in1=exv_t[:, :, CV:CD]
                        .unsqueeze(3)
                        .to_broadcast([TILE, etc, H, 24]),
                        op=mybir.AluOpType.mult,
                    )

                    # scatter: feat[n, c] += sum_e oh[e, n] * exv[e, c]
                    for eti in range(etc):
                        nc.tensor.matmul(
                            feat_p[:],
                            oh_t[:, eti, :],
                            exv_t[:, eti, :],
                            start=(ch == 0 and eti == 0),
                            stop=(ch == nch - 1 and eti == etc - 1),
                        )

                feat_s = outp.tile([TILE, CD], fp32, tag="feat")
                nc.scalar.activation(
                    feat_s[:], feat_p[:], mybir.ActivationFunctionType.Copy
                )
                nc.sync.dma_start(feat_d[nt], feat_s[:])

    if compile:
        nc.compile()
    return nc


def _build_nc_v2(nt_count=NT, et=ET, etc=ETC, compile=True):
    """V2: q[dst] gathered ON DEVICE via PE (oh^T transpose + one-hot matmul
    against the node-tile's 128 queries) instead of a host-gathered qg input.
    Cuts input DMA by ~44.6MB/core (qg_r) at the cost of PE/ACT work."""
    import concourse.bass as bass
    import concourse.tile as tile
    from concourse import bacc, mybir
    from concourse.masks import make_identity

    nch = et // etc
    assert nch * etc == et
    fp32 = mybir.dt.float32
    nc = bacc.Bacc(
        "TRN2",
        target_bir_lowering=False,
        debug=False,
        enable_asserts=False,
        num_devices=NCORES,
    )

    key_d = nc.dram_tensor("key_r", [nt_count, TILE, et, F], fp32, kind="ExternalInput").ap()
    q_d = nc.dram_tensor("q_r", [nt_count, TILE, F], fp32, kind="ExternalInput").ap()
    val_d = nc.dram_tensor("val_r", [nt_count, TILE, et, CV], fp32, kind="ExternalInput").ap()
    rel_d = nc.dram_tensor("rel_r", [nt_count, TILE, et], fp32, kind="ExternalInput").ap()
    feat_d = nc.dram_tensor("feat_r", [nt_count, TILE, CD], fp32, kind="ExternalOutput").ap()
    prelog_d = nc.dram_tensor(
        "prelog_r", [nt_count, TILE, et, H], fp32, kind="ExternalOutput"
    ).ap()

    # eti groups of <=4 share one PSUM qg tile ([128, 512] = 1 bank)
    groups = []
    g0 = 0
    while g0 < etc:
        gs = min(4, etc - g0)
        groups.append((g0, gs))
        g0 += gs

    with tile.TileContext(nc) as tc:
        with (
            tc.tile_pool(name="consts", bufs=1) as consts,
            tc.tile_pool(name="io", bufs=3) as io,
            tc.tile_pool(name="qio", bufs=2) as qio,
            tc.tile_pool(name="work", bufs=3) as work,
            tc.tile_pool(name="oh2p", bufs=4) as oh2p,
            tc.tile_pool(name="outp", bufs=2) as outp,
            tc.tile_pool(name="pfeat", bufs=2, space="PSUM") as pfeat,
            tc.tile_pool(name="ptrans", bufs=2, space="PSUM") as ptrans,
            tc.tile_pool(name="pqg", bufs=2, space="PSUM") as pqg,
        ):
            iota_t = consts.tile([TILE, TILE], fp32)
            nc.gpsimd.iota(
                iota_t[:],
                [[1, TILE]],
                channel_multiplier=0,
                allow_small_or_imprecise_dtypes=True,
            )
            ident_t = consts.tile([TILE, TILE], fp32)
            make_identity(nc, ident_t[:])

            for nt in range(nt_count):
                feat_p = pfeat.tile([TILE, CD], fp32)
                q_t = qio.tile([TILE, F], fp32, tag="q")
                nc.sync.dma_start(q_t[:], q_d[nt])
                for ch in range(nch):
                    et0 = ch * etc
                    k_t = io.tile([TILE, etc, F], fp32, tag="k")
                    nc.sync.dma_start(k_t[:], key_d[nt, :, et0 : et0 + etc, :])
                    v_t = io.tile([TILE, etc, CV], fp32, tag="v")
                    nc.sync.dma_start(v_t[:], val_d[nt, :, et0 : et0 + etc, :])
                    rel_t = io.tile([TILE, etc], fp32, tag="rel")
                    nc.sync.dma_start(rel_t[:], rel_d[nt, :, et0 : et0 + etc])

                    oh_t = work.tile([TILE, etc, TILE], fp32, tag="oh")
                    nc.vector.tensor_tensor(
                        out=oh_t[:],
                        in0=iota_t[:].unsqueeze(1).to_broadcast([TILE, etc, TILE]),
                        in1=rel_t[:].unsqueeze(2).to_broadcast([TILE, etc, TILE]),
                        op=mybir.AluOpType.is_equal,
                    )

                    # on-device q gather: qg[e, f] = sum_n oh[e, n] * q[n, f]
                    for g0, gs in groups:
                        qg_p = pqg.tile([TILE, 4 * TILE], fp32, tag="qg")
                        for j in range(gs):
                            eti = g0 + j
                            ohT_p = ptrans.tile([TILE, TILE], fp32, tag="ohT")
                            nc.tensor.transpose(
                                ohT_p[:], oh_t[:, eti, :], ident_t[:]
                            )
                            oh2_s = oh2p.tile([TILE, TILE], fp32, tag="oh2")
                            nc.scalar.activation(
                                oh2_s[:],
                                ohT_p[:],
                                mybir.ActivationFunctionType.Copy,
                            )
                            nc.tensor.matmul(
                                qg_p[:, j * TILE : (j + 1) * TILE],
                                oh2_s[:],
                                q_t[:],
                                start=True,
                                stop=True,
                            )
                        # kq = k * qg (in place over k), one op per group
                        nc.vector.tensor_tensor(
                            out=k_t[:, g0 : g0 + gs, :],
                            in0=k_t[:, g0 : g0 + gs, :],
                            in1=qg_p[:, : gs * TILE].rearrange(
                                "p (e f) -> p e f", f=F
                            ),
                            op=mybir.AluOpType.mult,
                        )

                    logit_t = work.tile([TILE, etc, H], fp32, tag="logit")
                    nc.vector.tensor_reduce(
                        out=logit_t[:],
                        in_=k_t[:].rearrange("p e (h d) -> p e h d", d=DHEAD),
                        axis=mybir.AxisListType.X,
                        op=mybir.AluOpType.add,
                    )

                    prelog_t = outp.tile([TILE, etc, H], fp32, tag="prelog")
                    nc.scalar.activation(
                        prelog_t[:],
                        logit_t[:],
                        mybir.ActivationFunctionType.Copy,
                        scale=SCALE,
                    )
                    nc.sync.dma_start(
                        prelog_d[nt, :, et0 : et0 + etc, :], prelog_t[:]
                    )

                    exv_t = work.tile([TILE, etc, CD], fp32, tag="exv")
                    nc.scalar.activation(
                        exv_t[:, :, CV:CD],
                        logit_t[:],
                        mybir.ActivationFunctionType.Exp,
                        scale=SCALE,
                    )
                    nc.gpsimd.tensor_tensor(
                        out=exv_t[:, :, 0:CV].rearrange("p e (h c) -> p e h c", c=24),
                        in0=v_t[:].rearrange("p e (h c) -> p e h c", c=24),
                        in1=exv_t[:, :, CV:CD]
                        .unsqueeze(3)
                        .to_broadcast([TILE, etc, H, 24]),
                        op=mybir.AluOpType.mult,
                    )

                    for eti in range(etc):
                        nc.tensor.matmul(
                            feat_p[:],
                            oh_t[:, eti, :],
                            exv_t[:, eti, :],
                            start=(ch == 0 and eti == 0),
                            stop=(ch == nch - 1 and eti == etc - 1),
                        )

                feat_s = outp.tile([TILE, CD], fp32, tag="feat")
                nc.scalar.activation(
                    feat_s[:], feat_p[:], mybir.ActivationFunctionType.Copy
                )
                nc.sync.dma_start(feat_d[nt], feat_s[:])

    if compile:
        nc.compile()
    return nc


USE_V2 = os.environ.get("KERNEL_V2", "1") == "1"


def _get_compiled():
    global _COMPILED
    if _COMPILED is None:
        _COMPILED = _build_nc_v2() if USE_V2 else _build_nc()
    return _COMPILED


def _host_prep(value, key_feat, query_0, query_1, dst):
    """Route edges to dst-owner cores, build padded per-core device inputs."""
    dst = np.asarray(dst).astype(np.int64)
    q_cat = np.concatenate(
        [np.asarray(query_0), np.asarray(query_1)], axis=-1
    ).reshape(N, F)
    key_feat = np.ascontiguousarray(np.asarray(key_feat, dtype=np.float32))
    val_flat = np.ascontiguousarray(
        np.asarray(value, dtype=np.float32).reshape(E, CV)
    )

    core_of = dst // NPC
    local = dst - core_of * NPC
    nt_of = local // TILE
    bucket = core_of * NT + nt_of
    order = np.argsort(bucket, kind="stable")
    counts = np.bincount(bucket, minlength=NCORES * NT).reshape(NCORES, NT)
    assert counts.max() <= SLOTS, f"padding overflow: {counts.max()} > {SLOTS}"

    starts = np.zeros(NCORES * NT + 1, dtype=np.int64)
    np.cumsum(counts.reshape(-1), out=starts[1:])

    in_maps = []
    perms = []
    rels = []
    for c in range(NCORES):
        perm = np.zeros((NT, SLOTS), dtype=np.int64)
        rel = np.full((NT, SLOTS), -1.0, dtype=np.float32)
        for t in range(NT):
            b = c * NT + t
            n_e = counts[c, t]
            sl = order[starts[b] : starts[b] + n_e]
            perm[t, :n_e] = sl
            rel[t, :n_e] = (local[sl] - t * TILE).astype(np.float32)
        perms.append(perm)
        rels.append(rel)

        key_r = (
            key_feat[perm]
            .reshape(NT, ET, TILE, F)
            .transpose(0, 2, 1, 3)
        )
        val_r = (
            val_flat[perm]
            .reshape(NT, ET, TILE, CV)
            .transpose(0, 2, 1, 3)
        )
        rel_r = rel.reshape(NT, ET, TILE).transpose(0, 2, 1)
        im = {
            "key_r": np.ascontiguousarray(key_r, dtype=np.float32),
            "val_r": np.ascontiguousarray(val_r, dtype=np.float32),
            "rel_r": np.ascontiguousarray(rel_r, dtype=np.float32),
        }
        if USE_V2:
            q_r = np.zeros((NT, TILE, F), dtype=np.float32)
            q_core = q_cat[c * NPC : (c + 1) * NPC]
            q_r.reshape(NT * TILE, F)[:NPC] = q_core
            im["q_r"] = q_r
        else:
            qg_r = (
                q_cat[dst[perm]]
                .reshape(NT, ET, TILE, F)
                .transpose(0, 2, 1, 3)
            )
            im["qg_r"] = np.ascontiguousarray(qg_r, dtype=np.float32)
        in_maps.append(im)
    return in_maps, perms, rels


def _host_unshard(results, perms, rels):
    prelogits = np.zeros((E, H), dtype=np.float32)
    feat_n = np.zeros((N, CD), dtype=np.float32)
    for c in range(NCORES):
        out = results[c]
        pr = (
            np.asarray(out["prelog_r"])
            .transpose(0, 2, 1, 3)
            .reshape(NT, SLOTS, H)
        )
        mask = rels[c] >= 0.0
        prelogits[perms[c][mask]] = pr[mask]
        fr = np.asarray(out["feat_r"]).reshape(NT * TILE, CD)
        feat_n[c * NPC : (c + 1) * NPC] = fr[:NPC]
    s = feat_n[:, CV:CD]
    unnorm = feat_n[:, :CV].reshape(N, H, 8, 3)
    feat = unnorm / s[:, :, None, None]
    feat = feat.reshape(N, 64, 3)
    out_0 = np.ascontiguousarray(feat[:, :32, :1])
    out_1 = np.ascontiguousarray(feat[:, 32:, :3])
    return out_0, out_1, prelogits


def _install_ntff_hook_shim():
    """The agent image's antenv lacks axon_hooks; recreate it so trace=True
    works under axon. Best-effort — degrades to no tracing on any failure."""
    import sys
    import types

    if "antenv.axon_hooks" in sys.modules:
        return
    try:
        from trn_agent_boot.trn_boot import _ntff_profile_via_ctypes

        hook = _ntff_profile_via_ctypes("/opt/axon/libaxon_pjrt.so")
        mod = types.ModuleType("antenv.axon_hooks")
        mod._hook = hook
        mod.set_axon_ntff_profile_hook = lambda h: setattr(mod, "_hook", h)
        mod.get_axon_ntff_profile_hook = lambda: mod._hook
        sys.modules["antenv.axon_hooks"] = mod
        import antenv

        antenv.axon_hooks = mod
    except Exception:
        pass


def kernel(value, key_feat, query_0, query_1, dst):
    global LAST_RESULTS
    from concourse import bass_utils

    nc = _get_compiled()
    in_maps, perms, rels = _host_prep(value, key_feat, query_0, query_1, dst)
    trace = os.environ.get("KERNEL_TRACE", "0") == "1"
    if trace:
        _install_ntff_hook_shim()
    res = bass_utils.run_bass_kernel_spmd(
        nc,
        in_maps,
        core_ids=list(range(NCORES)),
        trace=trace,
    )
    LAST_RESULTS = res
    return _host_unshard(res.results, perms, rels)


# revision 13
# speedup vs baseline: 1.0990x; 1.0990x over previous
"""AttentionSE3 message-passing kernel for 8 Trainium2 NeuronCores.

Strategy (node-sharded, zero device collectives):
  - The softmax over incoming edges of each dst node is computed WITHOUT the
    max-subtraction (prelogits are ~N(0, 0.35^2) so exp never overflows), and
    the division by the softmax denominator commutes with the segment-sum:
        feat[n] = (sum_{e->n} exp(pl_e) * v_e) / (sum_{e->n} exp(pl_e))
    so each core only needs the edges whose dst it owns.
  - Host: sort edges by dst, route each edge to the core owning its dst node
    (2500 nodes/core), group into 20 node-tiles of 128 nodes, pad each
    node-tile's edge list to ET*128 slots. Gather key/value/q[dst] rows into
    the padded layout.
  - Device (per core): for each node-tile, build a 0/1 one-hot matrix
    oh[e, n] = (rel[e] == n) on the VectorE, compute per-edge/per-head logits
    as an elementwise mul + grouped reduce, exp on ScalarE, then segment-sum
    s and exp*value with a single TensorE matmul  feat = oh^T @ [exv | ex].
  - Host: un-permute prelogits, divide feat by s, slice outputs.
"""

import math
import os

import numpy as np

# ---- problem constants (hardcoded; must match reference.setup_inputs) ----
N = 20000
E = 640000
H = 8
DHEAD = 16
F = 128            # NUM_FEATURES = H * DHEAD
CV = 192           # value channels flattened (64*3), layout [8h, 8c, 3d]
CD = CV + H        # 200: value channels + per-head ex column for s
NCORES = 8
NPC = N // NCORES  # 2500 nodes per core
TILE = 128
NT = 20            # node tiles per core  (20*128 = 2560 >= 2500)
ET = 34            # max edge tiles per node tile (max count 4252 <= 4352)
SLOTS = ET * TILE
ETC = 17           # edge tiles per chunk
NCH = ET // ETC    # 2
SCALE = 1.0 / math.sqrt(float(F))

_COMPILED = None
LAST_RESULTS = None


def _build_nc(nt_count=NT, et=ET, etc=ETC, compile=True, use_fp32r=True):
    import concourse.bass as bass
    import concourse.tile as tile
    from concourse import bacc, mybir

    nch = et // etc
    assert nch * etc == et

    fp32 = mybir.dt.float32
    fmm = mybir.dt.float32r if use_fp32r else mybir.dt.float32
    # fp32r fast path needs matmul output free size >= 256
    CDP = 256 if use_fp32r else CD
    nc = bacc.Bacc(
        "TRN2",
        target_bir_lowering=False,
        debug=False,
        enable_asserts=False,
        num_devices=NCORES,
    )

    key_d = nc.dram_tensor("key_r", [nt_count, TILE, et, F], fp32, kind="ExternalInput").ap()
    qg_d = nc.dram_tensor("qg_r", [nt_count, TILE, et, F], fp32, kind="ExternalInput").ap()
    val_d = nc.dram_tensor("val_r", [nt_count, TILE, et, CV], fp32, kind="ExternalInput").ap()
    rel_d = nc.dram_tensor("rel_r", [nt_count, TILE, et], fp32, kind="ExternalInput").ap()
    feat_d = nc.dram_tensor("feat_r", [nt_count, TILE, CD], fp32, kind="ExternalOutput").ap()
    prelog_d = nc.dram_tensor(
        "prelog_r", [nt_count, TILE, et, H], fp32, kind="ExternalOutput"
    ).ap()

    with tile.TileContext(nc) as tc:
        with (
            tc.tile_pool(name="consts", bufs=1) as consts,
            tc.tile_pool(name="io", bufs=3) as io,
            tc.tile_pool(name="work", bufs=3) as work,
            tc.tile_pool(name="outp", bufs=2) as outp,
            tc.tile_pool(name="pfeat", bufs=2, space="PSUM") as pfeat,
        ):
            iota_t = consts.tile([TILE, TILE], fp32)
            nc.gpsimd.iota(
                iota_t[:],
                [[1, TILE]],
                channel_multiplier=0,
                allow_small_or_imprecise_dtypes=True,
            )

            for nt in range(nt_count):
                feat_p = pfeat.tile([TILE, CDP], fp32)
                for ch in range(nch):
                    et0 = ch * etc
                    k_t = io.tile([TILE, etc, F], fp32, tag="k")
                    nc.sync.dma_start(k_t[:], key_d[nt, :, et0 : et0 + etc, :])
                    qg_t = io.tile([TILE, etc, F], fp32, tag="qg")
                    nc.sync.dma_start(qg_t[:], qg_d[nt, :, et0 : et0 + etc, :])
                    v_t = io.tile([TILE, etc, CV], fp32, tag="v")
                    nc.sync.dma_start(v_t[:], val_d[nt, :, et0 : et0 + etc, :])
                    rel_t = io.tile([TILE, etc], fp32, tag="rel")
                    nc.sync.dma_start(rel_t[:], rel_d[nt, :, et0 : et0 + etc])

                    # one-hot: oh[p, et, j] = (rel[p, et] == j)
                    oh_t = work.tile([TILE, etc, TILE], fmm, tag="oh")
                    nc.vector.tensor_tensor(
                        out=oh_t[:],
                        in0=iota_t[:].unsqueeze(1).to_broadcast([TILE, etc, TILE]),
                        in1=rel_t[:].unsqueeze(2).to_broadcast([TILE, etc, TILE]),
                        op=mybir.AluOpType.is_equal,
                    )

                    # kq = k * q_gathered (in place over k)
                    nc.vector.tensor_tensor(
                        out=k_t[:], in0=k_t[:], in1=qg_t[:], op=mybir.AluOpType.mult
                    )
                    # logits[p, et, h] = sum_d kq[p, et, h, d]
                    logit_t = work.tile([TILE, etc, H], fp32, tag="logit")
                    nc.vector.tensor_reduce(
                        out=logit_t[:],
                        in_=k_t[:].rearrange("p e (h d) -> p e h d", d=DHEAD),
                        axis=mybir.AxisListType.X,
                        op=mybir.AluOpType.add,
                    )

                    # prelogits output = raw logits (host multiplies by SCALE)
                    nc.sync.dma_start(
                        prelog_d[nt, :, et0 : et0 + etc, :], logit_t[:]
                    )

                    # exv[:, :, 192:200] = exp(logits * SCALE)
                    exv_t = work.tile([TILE, etc, CDP], fmm, tag="exv")
                    nc.scalar.activation(
                        exv_t[:, :, CV:CD],
                        logit_t[:],
                        mybir.ActivationFunctionType.Exp,
                        scale=SCALE,
                    )
                    # exv[:, :, 0:192] = v * ex (broadcast ex over 24 channels)
                    # on GPSIMD: DVE is the critical engine, GpSimd is idle
                    nc.gpsimd.tensor_tensor(
                        out=exv_t[:, :, 0:CV].rearrange("p e (h c) -> p e h c", c=24),
                        in0=v_t[:].rearrange("p e (h c) -> p e h c", c=24),
                        in1=exv_t[:, :, CV:CD]
                        .unsqueeze(3)
                        .to_broadcast([TILE, etc, H, 24]),
                        op=mybir.AluOpType.mult,
                    )
                    # (pad columns CD:CDP are left uninitialized; any garbage
                    # lands only in feat columns >= CD which the host ignores)

                    # scatter: feat[n, c] += sum_e oh[e, n] * exv[e, c]
                    for eti in range(etc):
                        nc.tensor.matmul(
                            feat_p[:],
                            oh_t[:, eti, :],
                            exv_t[:, eti, :],
                            start=(ch == 0 and eti == 0),
                            stop=(ch == nch - 1 and eti == etc - 1),
                        )

                feat_s = outp.tile([TILE, CD], fp32, tag="feat")
                nc.vector.tensor_copy(feat_s[:], feat_p[:, :CD])
                nc.sync.dma_start(feat_d[nt], feat_s[:])

    if compile:
        nc.compile()
    return nc


def _build_nc_v2(nt_count=NT, et=ET, etc=ETC, compile=True):
    """V2: q[dst] gathered ON DEVICE via PE (oh^T transpose + one-hot matmul
    against the node-tile's 128 queries) instead of a host-gathered qg input.
    Cuts input DMA by ~44.6MB/core (qg_r) at the cost of PE/ACT work."""
    import concourse.bass as bass
    import concourse.tile as tile
    from concourse import bacc, mybir
    from concourse.masks import make_identity

    nch = et // etc
    assert nch * etc == et
    fp32 = mybir.dt.float32
    nc = bacc.Bacc(
        "TRN2",
        target_bir_lowering=False,
        debug=False,
        enable_asserts=False,
        num_devices=NCORES,
    )

    key_d = nc.dram_tensor("key_r", [nt_count, TILE, et, F], fp32, kind="ExternalInput").ap()
    q_d = nc.dram_tensor("q_r", [nt_count, TILE, F], fp32, kind="ExternalInput").ap()
    val_d = nc.dram_tensor("val_r", [nt_count, TILE, et, CV], fp32, kind="ExternalInput").ap()
    rel_d = nc.dram_tensor("rel_r", [nt_count, TILE, et], fp32, kind="ExternalInput").ap()
    feat_d = nc.dram_tensor("feat_r", [nt_count, TILE, CD], fp32, kind="ExternalOutput").ap()
    prelog_d = nc.dram_tensor(
        "prelog_r", [nt_count, TILE, et, H], fp32, kind="ExternalOutput"
    ).ap()

    # eti groups of <=4 share one PSUM qg tile ([128, 512] = 1 bank)
    groups = []
    g0 = 0
    while g0 < etc:
        gs = min(4, etc - g0)
        groups.append((g0, gs))
        g0 += gs

    with tile.TileContext(nc) as tc:
        with (
            tc.tile_pool(name="consts", bufs=1) as consts,
            tc.tile_pool(name="io", bufs=3) as io,
            tc.tile_pool(name="qio", bufs=2) as qio,
            tc.tile_pool(name="work", bufs=3) as work,
            tc.tile_pool(name="oh2p", bufs=4) as oh2p,
            tc.tile_pool(name="outp", bufs=2) as outp,
            tc.tile_pool(name="pfeat", bufs=2, space="PSUM") as pfeat,
            tc.tile_pool(name="ptrans", bufs=2, space="PSUM") as ptrans,
            tc.tile_pool(name="pqg", bufs=2, space="PSUM") as pqg,
        ):
            iota_t = consts.tile([TILE, TILE], fp32)
            nc.gpsimd.iota(
                iota_t[:],
                [[1, TILE]],
                channel_multiplier=0,
                allow_small_or_imprecise_dtypes=True,
            )
            ident_t = consts.tile([TILE, TILE], fp32)
            make_identity(nc, ident_t[:])

            for nt in range(nt_count):
                feat_p = pfeat.tile([TILE, CD], fp32)
                q_t = qio.tile([TILE, F], fp32, tag="q")
                nc.sync.dma_start(q_t[:], q_d[nt])
                for ch in range(nch):
                    et0 = ch * etc
                    k_t = io.tile([TILE, etc, F], fp32, tag="k")
                    nc.sync.dma_start(k_t[:], key_d[nt, :, et0 : et0 + etc, :])
                    v_t = io.tile([TILE, etc, CV], fp32, tag="v")
                    nc.sync.dma_start(v_t[:], val_d[nt, :, et0 : et0 + etc, :])
                    rel_t = io.tile([TILE, etc], fp32, tag="rel")
                    nc.sync.dma_start(rel_t[:], rel_d[nt, :, et0 : et0 + etc])

                    oh_t = work.tile([TILE, etc, TILE], fp32, tag="oh")
                    nc.vector.tensor_tensor(
                        out=oh_t[:],
                        in0=iota_t[:].unsqueeze(1).to_broadcast([TILE, etc, TILE]),
                        in1=rel_t[:].unsqueeze(2).to_broadcast([TILE, etc, TILE]),
                        op=mybir.AluOpType.is_equal,
                    )

                    # on-device q gather: qg[e, f] = sum_n oh[e, n] * q[n, f]
                    for g0, gs in groups:
                        qg_p = pqg.tile([TILE, 4 * TILE], fp32, tag="qg")
                        for j in range(gs):
                            eti = g0 + j
                            ohT_p = ptrans.tile([TILE, TILE], fp32, tag="ohT")
                            nc.tensor.transpose(
                                ohT_p[:], oh_t[:, eti, :], ident_t[:]
                            )
                            oh2_s = oh2p.tile([TILE, TILE], fp32, tag="oh2")
                            nc.scalar.activation(
                                oh2_s[:],
                                ohT_p[:],
                                mybir.ActivationFunctionType.Copy,
                            )
                            nc.tensor.matmul(
                                qg_p[:, j * TILE : (j + 1) * TILE],
                                oh2_s[:],
                                q_t[:],
                                start=True,
                                stop=True,
                            )
                        # kq = k * qg (in place over k), one op per group
                        nc.vector.tensor_tensor(
                            out=k_t[:, g0 : g0 + gs, :],
                            in0=k_t[:, g0 : g0 + gs, :],
                            in1=qg_p[:, : gs * TILE].rearrange(
                                "p (e f) -> p e f", f=F
                            ),
                            op=mybir.AluOpType.mult,
                        )

                    logit_t = work.tile([TILE, etc, H], fp32, tag="logit")
                    nc.vector.tensor_reduce(
                        out=logit_t[:],
                        in_=k_t[:].rearrange("p e (h d) -> p e h d", d=DHEAD),
                        axis=mybir.AxisListType.X,
                        op=mybir.AluOpType.add,
                    )

                    nc.sync.dma_start(
                        prelog_d[nt, :, et0 : et0 + etc, :], logit_t[:]
                    )

                    exv_t = work.tile([TILE, etc, CD], fp32, tag="exv")
                    nc.scalar.activation(
                        exv_t[:, :, CV:CD],
                        logit_t[:],
                        mybir.ActivationFunctionType.Exp,
                        scale=SCALE,
                    )
                    nc.gpsimd.tensor_tensor(
                        out=exv_t[:, :, 0:CV].rearrange("p e (h c) -> p e h c", c=24),
                        in0=v_t[:].rearrange("p e (h c) -> p e h c", c=24),
                        in1=exv_t[:, :, CV:CD]
                        .unsqueeze(3)
                        .to_broadcast([TILE, etc, H, 24]),
                        op=mybir.AluOpType.mult,
                    )

                    for eti in range(etc):
                        nc.tensor.matmul(
                            feat_p[:],
                            oh_t[:, eti, :],
                            exv_t[:, eti, :],
                            start=(ch == 0 and eti == 0),
                            stop=(ch == nch - 1 and eti == etc - 1),
                        )

                feat_s = outp.tile([TILE, CD], fp32, tag="feat")
                nc.scalar.activation(
                    feat_s[:], feat_p[:], mybir.ActivationFunctionType.Copy
                )
                nc.sync.dma_start(feat_d[nt], feat_s[:])

    if compile:
        nc.compile()
    return nc


USE_V2 = os.environ.get("KERNEL_V2", "0") == "1"


def _get_compiled():
    global _COMPILED
    if _COMPILED is None:
        _COMPILED = _build_nc_v2() if USE_V2 else _build_nc()
    return _COMPILED


def _host_prep(value, key_feat, query_0, query_1, dst):
    """Route edges to dst-owner cores, build padded per-core device inputs."""
    dst = np.asarray(dst).astype(np.int64)
    q_cat = np.concatenate(
        [np.asarray(query_0), np.asarray(query_1)], axis=-1
    ).reshape(N, F)
    key_feat = np.ascontiguousarray(np.asarray(key_feat, dtype=np.float32))
    val_flat = np.ascontiguousarray(
        np.asarray(value, dtype=np.float32).reshape(E, CV)
    )

    core_of = dst // NPC
    local = dst - core_of * NPC
    nt_of = local // TILE
    bucket = core_of * NT + nt_of
    order = np.argsort(bucket, kind="stable")
    counts = np.bincount(bucket, minlength=NCORES * NT).reshape(NCORES, NT)
    assert counts.max() <= SLOTS, f"padding overflow: {counts.max()} > {SLOTS}"

    starts = np.zeros(NCORES * NT + 1, dtype=np.int64)
    np.cumsum(counts.reshape(-1), out=starts[1:])

    in_maps = []
    perms = []
    rels = []
    for c in range(NCORES):
        perm = np.zeros((NT, SLOTS), dtype=np.int64)
        rel = np.full((NT, SLOTS), -1.0, dtype=np.float32)
        for t in range(NT):
            b = c * NT + t
            n_e = counts[c, t]
            sl = order[starts[b] : starts[b] + n_e]
            perm[t, :n_e] = sl
            rel[t, :n_e] = (local[sl] - t * TILE).astype(np.float32)
        perms.append(perm)
        rels.append(rel)

        key_r = (
            key_feat[perm]
            .reshape(NT, ET, TILE, F)
            .transpose(0, 2, 1, 3)
        )
        val_r = (
            val_flat[perm]
            .reshape(NT, ET, TILE, CV)
            .transpose(0, 2, 1, 3)
        )
        rel_r = rel.reshape(NT, ET, TILE).transpose(0, 2, 1)
        im = {
            "key_r": np.ascontiguousarray(key_r, dtype=np.float32),
            "val_r": np.ascontiguousarray(val_r, dtype=np.float32),
            "rel_r": np.ascontiguousarray(rel_r, dtype=np.float32),
        }
        if USE_V2:
            q_r = np.zeros((NT, TILE, F), dtype=np.float32)
            q_core = q_cat[c * NPC : (c + 1) * NPC]
            q_r.reshape(NT * TILE, F)[:NPC] = q_core
            im["q_r"] = q_r
        else:
            qg_r = (
                q_cat[dst[perm]]
                .reshape(NT, ET, TILE, F)
                .transpose(0, 2, 1, 3)
            )
            im["qg_r"] = np.ascontiguousarray(qg_r, dtype=np.float32)
        in_maps.append(im)
    return in_maps, perms, rels


def _host_unshard(results, perms, rels):
    prelogits = np.zeros((E, H), dtype=np.float32)
    feat_n = np.zeros((N, CD), dtype=np.float32)
    for c in range(NCORES):
        out = results[c]
        pr = (
            np.asarray(out["prelog_r"])
            .transpose(0, 2, 1, 3)
            .reshape(NT, SLOTS, H)
        )
        mask = rels[c] >= 0.0
        prelogits[perms[c][mask]] = pr[mask] * SCALE
        fr = np.asarray(out["feat_r"]).reshape(NT * TILE, CD)
        feat_n[c * NPC : (c + 1) * NPC] = fr[:NPC]
    s = feat_n[:, CV:CD]
    unnorm = feat_n[:, :CV].reshape(N, H, 8, 3)
    feat = unnorm / s[:, :, None, None]
    feat = feat.reshape(N, 64, 3)
    out_0 = np.ascontiguousarray(feat[:, :32, :1])
    out_1 = np.ascontiguousarray(feat[:, 32:, :3])
    return out_0, out_1, prelogits


def _install_ntff_hook_shim():
    """The agent image's antenv lacks axon_hooks; recreate it so trace=True
    works under axon. Best-effort — degrades to no tracing on any failure."""
    import sys
    import types

    if "antenv.axon_hooks" in sys.modules:
        return
    try:
        from trn_agent_boot.trn_boot import _ntff_profile_via_ctypes

        hook = _ntff_profile_via_ctypes("/opt/axon/libaxon_pjrt.so")
        mod = types.ModuleType("antenv.axon_hooks")
        mod._hook = hook
        mod.set_axon_ntff_profile_hook = lambda h: setattr(mod, "_hook", h)
        mod.get_axon_ntff_profile_hook = lambda: mod._hook
        sys.modules["antenv.axon_hooks"] = mod
        import antenv

        antenv.axon_hooks = mod
    except Exception:
        pass


def kernel(value, key_feat, query_0, query_1, dst):
    global LAST_RESULTS
    from concourse import bass_utils

    nc = _get_compiled()
    in_maps, perms, rels = _host_prep(value, key_feat, query_0, query_1, dst)
    trace = os.environ.get("KERNEL_TRACE", "0") == "1"
    if trace:
        _install_ntff_hook_shim()
    res = bass_utils.run_bass_kernel_spmd(
        nc,
        in_maps,
        core_ids=list(range(NCORES)),
        trace=trace,
    )
    LAST_RESULTS = res
    return _host_unshard(res.results, perms, rels)


# revision 15
# speedup vs baseline: 1.4325x; 1.3035x over previous
"""AttentionSE3 message-passing kernel for 8 Trainium2 NeuronCores.

Strategy (node-sharded, zero device collectives):
  - The softmax over incoming edges of each dst node is computed WITHOUT the
    max-subtraction (prelogits are ~N(0, 0.35^2) so exp never overflows), and
    the division by the softmax denominator commutes with the segment-sum:
        feat[n] = (sum_{e->n} exp(pl_e) * v_e) / (sum_{e->n} exp(pl_e))
    so each core only needs the edges whose dst it owns.
  - Host: sort edges by dst, route each edge to the core owning its dst node
    (2500 nodes/core), group into 20 node-tiles of 128 nodes, pad each
    node-tile's edge list to ET*128 slots. Gather key/value/q[dst] rows into
    the padded layout.
  - Device (per core): for each node-tile, build a 0/1 one-hot matrix
    oh[e, n] = (rel[e] == n) on the VectorE, compute per-edge/per-head logits
    as an elementwise mul + grouped reduce, exp on ScalarE, then segment-sum
    s and exp*value with a single TensorE matmul  feat = oh^T @ [exv | ex].
  - Host: un-permute prelogits, divide feat by s, slice outputs.
"""

import math
import os

import numpy as np

# ---- problem constants (hardcoded; must match reference.setup_inputs) ----
N = 20000
E = 640000
H = 8
DHEAD = 16
F = 128            # NUM_FEATURES = H * DHEAD
CV = 192           # value channels flattened (64*3), layout [8h, 8c, 3d]
CD = CV + H        # 200: value channels + per-head ex column for s
NCORES = 8
NPC = N // NCORES  # 2500 nodes per core
TILE = 128
NT = 20            # node tiles per core  (20*128 = 2560 >= 2500)
ET = 34            # max edge tiles per node tile (max count 4252 <= 4352)
SLOTS = ET * TILE
ETC = 17           # edge tiles per chunk
NCH = ET // ETC    # 2
SCALE = 1.0 / math.sqrt(float(F))

_COMPILED = None
LAST_RESULTS = None


def _build_nc(nt_count=NT, et=ET, etc=ETC, compile=True, mode="fp16"):
    import concourse.bass as bass
    import concourse.tile as tile
    from concourse import bacc, mybir

    nch = et // etc
    assert nch * etc == et

    fp32 = mybir.dt.float32
    fp16 = mybir.dt.float16
    # dtype of the matmul operands (one-hot lhsT + exv rhs) and of the
    # key/qg/val transport
    if mode == "fp32r":
        fmm, fio = mybir.dt.float32r, fp32
        CDP = 256  # fp32r fast path needs matmul output free size >= 256
    elif mode == "fp16":
        fmm, fio = fp16, fp16
        CDP = CD
    else:
        fmm, fio = fp32, fp32
        CDP = CD
    nc = bacc.Bacc(
        "TRN2",
        target_bir_lowering=False,
        debug=False,
        enable_asserts=False,
        num_devices=NCORES,
    )

    key_d = nc.dram_tensor("key_r", [nt_count, TILE, et, F], fio, kind="ExternalInput").ap()
    qg_d = nc.dram_tensor("qg_r", [nt_count, TILE, et, F], fio, kind="ExternalInput").ap()
    val_d = nc.dram_tensor("val_r", [nt_count, TILE, et, CV], fio, kind="ExternalInput").ap()
    rel_d = nc.dram_tensor("rel_r", [nt_count, TILE, et], fp32, kind="ExternalInput").ap()
    feat_d = nc.dram_tensor("feat_r", [nt_count, TILE, CD], fp32, kind="ExternalOutput").ap()
    prelog_d = nc.dram_tensor(
        "prelog_r", [nt_count, TILE, et, H], fp32, kind="ExternalOutput"
    ).ap()

    with tile.TileContext(nc) as tc:
        with (
            tc.tile_pool(name="consts", bufs=1) as consts,
            tc.tile_pool(name="io", bufs=3) as io,
            tc.tile_pool(name="work", bufs=3) as work,
            tc.tile_pool(name="outp", bufs=2) as outp,
            tc.tile_pool(name="pfeat", bufs=2, space="PSUM") as pfeat,
        ):
            iota_t = consts.tile([TILE, TILE], fp32)
            nc.gpsimd.iota(
                iota_t[:],
                [[1, TILE]],
                channel_multiplier=0,
                allow_small_or_imprecise_dtypes=True,
            )

            for nt in range(nt_count):
                feat_p = pfeat.tile([TILE, CDP], fp32)
                for ch in range(nch):
                    et0 = ch * etc
                    k_t = io.tile([TILE, etc, F], fio, tag="k")
                    nc.sync.dma_start(k_t[:], key_d[nt, :, et0 : et0 + etc, :])
                    qg_t = io.tile([TILE, etc, F], fio, tag="qg")
                    nc.sync.dma_start(qg_t[:], qg_d[nt, :, et0 : et0 + etc, :])
                    v_t = io.tile([TILE, etc, CV], fio, tag="v")
                    nc.sync.dma_start(v_t[:], val_d[nt, :, et0 : et0 + etc, :])
                    rel_t = io.tile([TILE, etc], fp32, tag="rel")
                    nc.sync.dma_start(rel_t[:], rel_d[nt, :, et0 : et0 + etc])

                    # one-hot: oh[p, et, j] = (rel[p, et] == j)
                    oh_t = work.tile([TILE, etc, TILE], fmm, tag="oh")
                    nc.vector.tensor_tensor(
                        out=oh_t[:],
                        in0=iota_t[:].unsqueeze(1).to_broadcast([TILE, etc, TILE]),
                        in1=rel_t[:].unsqueeze(2).to_broadcast([TILE, etc, TILE]),
                        op=mybir.AluOpType.is_equal,
                    )

                    # kq = k * q_gathered, in fp32 (in place over k when fp32)
                    if mode == "fp16":
                        kq_t = work.tile([TILE, etc, F], fp32, tag="kq")
                    else:
                        kq_t = k_t
                    nc.vector.tensor_tensor(
                        out=kq_t[:], in0=k_t[:], in1=qg_t[:], op=mybir.AluOpType.mult
                    )
                    # logits[p, et, h] = sum_d kq[p, et, h, d]
                    logit_t = work.tile([TILE, etc, H], fp32, tag="logit")
                    nc.vector.tensor_reduce(
                        out=logit_t[:],
                        in_=kq_t[:].rearrange("p e (h d) -> p e h d", d=DHEAD),
                        axis=mybir.AxisListType.X,
                        op=mybir.AluOpType.add,
                    )

                    # prelogits output = raw logits (host multiplies by SCALE)
                    nc.sync.dma_start(
                        prelog_d[nt, :, et0 : et0 + etc, :], logit_t[:]
                    )

                    # exv[:, :, 192:200] = exp(logits * SCALE)
                    exv_t = work.tile([TILE, etc, CDP], fmm, tag="exv")
                    nc.scalar.activation(
                        exv_t[:, :, CV:CD],
                        logit_t[:],
                        mybir.ActivationFunctionType.Exp,
                        scale=SCALE,
                    )
                    # exv[:, :, 0:192] = v * ex (broadcast ex over 24 channels)
                    # split across GpSimd/DVE to balance engine load
                    exv_eng = nc.gpsimd if (mode != "fp16" or ch % 2 == 0) else nc.vector
                    exv_eng.tensor_tensor(
                        out=exv_t[:, :, 0:CV].rearrange("p e (h c) -> p e h c", c=24),
                        in0=v_t[:].rearrange("p e (h c) -> p e h c", c=24),
                        in1=exv_t[:, :, CV:CD]
                        .unsqueeze(3)
                        .to_broadcast([TILE, etc, H, 24]),
                        op=mybir.AluOpType.mult,
                    )
                    # (fp32r pad columns CD:CDP are left uninitialized; garbage
                    # lands only in feat columns >= CD which the host ignores)

                    # scatter: feat[n, c] += sum_e oh[e, n] * exv[e, c]
                    for eti in range(etc):
                        nc.tensor.matmul(
                            feat_p[:],
                            oh_t[:, eti, :],
                            exv_t[:, eti, :],
                            start=(ch == 0 and eti == 0),
                            stop=(ch == nch - 1 and eti == etc - 1),
                        )

                feat_s = outp.tile([TILE, CD], fp32, tag="feat")
                nc.vector.tensor_copy(feat_s[:], feat_p[:, :CD])
                nc.sync.dma_start(feat_d[nt], feat_s[:])

    if compile:
        nc.compile()
    return nc


def _build_nc_v2(nt_count=NT, et=ET, etc=ETC, compile=True):
    """V2: q[dst] gathered ON DEVICE via PE (oh^T transpose + one-hot matmul
    against the node-tile's 128 queries) instead of a host-gathered qg input.
    Cuts input DMA by ~44.6MB/core (qg_r) at the cost of PE/ACT work."""
    import concourse.bass as bass
    import concourse.tile as tile
    from concourse import bacc, mybir
    from concourse.masks import make_identity

    nch = et // etc
    assert nch * etc == et
    fp32 = mybir.dt.float32
    nc = bacc.Bacc(
        "TRN2",
        target_bir_lowering=False,
        debug=False,
        enable_asserts=False,
        num_devices=NCORES,
    )

    key_d = nc.dram_tensor("key_r", [nt_count, TILE, et, F], fp32, kind="ExternalInput").ap()
    q_d = nc.dram_tensor("q_r", [nt_count, TILE, F], fp32, kind="ExternalInput").ap()
    val_d = nc.dram_tensor("val_r", [nt_count, TILE, et, CV], fp32, kind="ExternalInput").ap()
    rel_d = nc.dram_tensor("rel_r", [nt_count, TILE, et], fp32, kind="ExternalInput").ap()
    feat_d = nc.dram_tensor("feat_r", [nt_count, TILE, CD], fp32, kind="ExternalOutput").ap()
    prelog_d = nc.dram_tensor(
        "prelog_r", [nt_count, TILE, et, H], fp32, kind="ExternalOutput"
    ).ap()

    # eti groups of <=4 share one PSUM qg tile ([128, 512] = 1 bank)
    groups = []
    g0 = 0
    while g0 < etc:
        gs = min(4, etc - g0)
        groups.append((g0, gs))
        g0 += gs

    with tile.TileContext(nc) as tc:
        with (
            tc.tile_pool(name="consts", bufs=1) as consts,
            tc.tile_pool(name="io", bufs=3) as io,
            tc.tile_pool(name="qio", bufs=2) as qio,
            tc.tile_pool(name="work", bufs=3) as work,
            tc.tile_pool(name="oh2p", bufs=4) as oh2p,
            tc.tile_pool(name="outp", bufs=2) as outp,
            tc.tile_pool(name="pfeat", bufs=2, space="PSUM") as pfeat,
            tc.tile_pool(name="ptrans", bufs=2, space="PSUM") as ptrans,
            tc.tile_pool(name="pqg", bufs=2, space="PSUM") as pqg,
        ):
            iota_t = consts.tile([TILE, TILE], fp32)
            nc.gpsimd.iota(
                iota_t[:],
                [[1, TILE]],
                channel_multiplier=0,
                allow_small_or_imprecise_dtypes=True,
            )
            ident_t = consts.tile([TILE, TILE], fp32)
            make_identity(nc, ident_t[:])

            for nt in range(nt_count):
                feat_p = pfeat.tile([TILE, CD], fp32)
                q_t = qio.tile([TILE, F], fp32, tag="q")
                nc.sync.dma_start(q_t[:], q_d[nt])
                for ch in range(nch):
                    et0 = ch * etc
                    k_t = io.tile([TILE, etc, F], fp32, tag="k")
                    nc.sync.dma_start(k_t[:], key_d[nt, :, et0 : et0 + etc, :])
                    v_t = io.tile([TILE, etc, CV], fp32, tag="v")
                    nc.sync.dma_start(v_t[:], val_d[nt, :, et0 : et0 + etc, :])
                    rel_t = io.tile([TILE, etc], fp32, tag="rel")
                    nc.sync.dma_start(rel_t[:], rel_d[nt, :, et0 : et0 + etc])

                    oh_t = work.tile([TILE, etc, TILE], fp32, tag="oh")
                    nc.vector.tensor_tensor(
                        out=oh_t[:],
                        in0=iota_t[:].unsqueeze(1).to_broadcast([TILE, etc, TILE]),
                        in1=rel_t[:].unsqueeze(2).to_broadcast([TILE, etc, TILE]),
                        op=mybir.AluOpType.is_equal,
                    )

                    # on-device q gather: qg[e, f] = sum_n oh[e, n] * q[n, f]
                    for g0, gs in groups:
                        qg_p = pqg.tile([TILE, 4 * TILE], fp32, tag="qg")
                        for j in range(gs):
                            eti = g0 + j
                            ohT_p = ptrans.tile([TILE, TILE], fp32, tag="ohT")
                            nc.tensor.transpose(
                                ohT_p[:], oh_t[:, eti, :], ident_t[:]
                            )
                            oh2_s = oh2p.tile([TILE, TILE], fp32, tag="oh2")
                            nc.scalar.activation(
                                oh2_s[:],
                                ohT_p[:],
                                mybir.ActivationFunctionType.Copy,
                            )
                            nc.tensor.matmul(
                                qg_p[:, j * TILE : (j + 1) * TILE],
                                oh2_s[:],
                                q_t[:],
                                start=True,
                                stop=True,
                            )
                        # kq = k * qg (in place over k), one op per group
                        nc.vector.tensor_tensor(
                            out=k_t[:, g0 : g0 + gs, :],
                            in0=k_t[:, g0 : g0 + gs, :],
                            in1=qg_p[:, : gs * TILE].rearrange(
                                "p (e f) -> p e f", f=F
                            ),
                            op=mybir.AluOpType.mult,
                        )

                    logit_t = work.tile([TILE, etc, H], fp32, tag="logit")
                    nc.vector.tensor_reduce(
                        out=logit_t[:],
                        in_=k_t[:].rearrange("p e (h d) -> p e h d", d=DHEAD),
                        axis=mybir.AxisListType.X,
                        op=mybir.AluOpType.add,
                    )

                    nc.sync.dma_start(
                        prelog_d[nt, :, et0 : et0 + etc, :], logit_t[:]
                    )

                    exv_t = work.tile([TILE, etc, CD], fp32, tag="exv")
                    nc.scalar.activation(
                        exv_t[:, :, CV:CD],
                        logit_t[:],
                        mybir.ActivationFunctionType.Exp,
                        scale=SCALE,
                    )
                    nc.gpsimd.tensor_tensor(
                        out=exv_t[:, :, 0:CV].rearrange("p e (h c) -> p e h c", c=24),
                        in0=v_t[:].rearrange("p e (h c) -> p e h c", c=24),
                        in1=exv_t[:, :, CV:CD]
                        .unsqueeze(3)
                        .to_broadcast([TILE, etc, H, 24]),
                        op=mybir.AluOpType.mult,
                    )

                    for eti in range(etc):
                        nc.tensor.matmul(
                            feat_p[:],
                            oh_t[:, eti, :],
                            exv_t[:, eti, :],
                            start=(ch == 0 and eti == 0),
                            stop=(ch == nch - 1 and eti == etc - 1),
                        )

                feat_s = outp.tile([TILE, CD], fp32, tag="feat")
                nc.scalar.activation(
                    feat_s[:], feat_p[:], mybir.ActivationFunctionType.Copy
                )
                nc.sync.dma_start(feat_d[nt], feat_s[:])

    if compile:
        nc.compile()
    return nc


USE_V2 = os.environ.get("KERNEL_V2", "0") == "1"
MODE = os.environ.get("KERNEL_MODE", "fp16")
IO_DTYPE = {"fp16": np.float16}.get(MODE, np.float32)


def _get_compiled():
    global _COMPILED
    if _COMPILED is None:
        _COMPILED = _build_nc_v2() if USE_V2 else _build_nc(mode=MODE)
    return _COMPILED


def _host_prep(value, key_feat, query_0, query_1, dst):
    """Route edges to dst-owner cores, build padded per-core device inputs."""
    dst = np.asarray(dst).astype(np.int64)
    q_cat = np.concatenate(
        [np.asarray(query_0), np.asarray(query_1)], axis=-1
    ).reshape(N, F)
    key_feat = np.ascontiguousarray(np.asarray(key_feat, dtype=np.float32))
    val_flat = np.ascontiguousarray(
        np.asarray(value, dtype=np.float32).reshape(E, CV)
    )

    core_of = dst // NPC
    local = dst - core_of * NPC
    nt_of = local // TILE
    bucket = core_of * NT + nt_of
    order = np.argsort(bucket, kind="stable")
    counts = np.bincount(bucket, minlength=NCORES * NT).reshape(NCORES, NT)
    assert counts.max() <= SLOTS, f"padding overflow: {counts.max()} > {SLOTS}"

    starts = np.zeros(NCORES * NT + 1, dtype=np.int64)
    np.cumsum(counts.reshape(-1), out=starts[1:])

    in_maps = []
    perms = []
    rels = []
    for c in range(NCORES):
        perm = np.zeros((NT, SLOTS), dtype=np.int64)
        rel = np.full((NT, SLOTS), -1.0, dtype=np.float32)
        for t in range(NT):
            b = c * NT + t
            n_e = counts[c, t]
            sl = order[starts[b] : starts[b] + n_e]
            perm[t, :n_e] = sl
            rel[t, :n_e] = (local[sl] - t * TILE).astype(np.float32)
        perms.append(perm)
        rels.append(rel)

        key_r = (
            key_feat[perm]
            .reshape(NT, ET, TILE, F)
            .transpose(0, 2, 1, 3)
        )
        val_r = (
            val_flat[perm]
            .reshape(NT, ET, TILE, CV)
            .transpose(0, 2, 1, 3)
        )
        rel_r = rel.reshape(NT, ET, TILE).transpose(0, 2, 1)
        im = {
            "key_r": np.ascontiguousarray(key_r, dtype=IO_DTYPE),
            "val_r": np.ascontiguousarray(val_r, dtype=IO_DTYPE),
            "rel_r": np.ascontiguousarray(rel_r, dtype=np.float32),
        }
        if USE_V2:
            q_r = np.zeros((NT, TILE, F), dtype=np.float32)
            q_core = q_cat[c * NPC : (c + 1) * NPC]
            q_r.reshape(NT * TILE, F)[:NPC] = q_core
            im["q_r"] = q_r
        else:
            qg_r = (
                q_cat[dst[perm]]
                .reshape(NT, ET, TILE, F)
                .transpose(0, 2, 1, 3)
            )
            im["qg_r"] = np.ascontiguousarray(qg_r, dtype=IO_DTYPE)
        in_maps.append(im)
    return in_maps, perms, rels


def _host_unshard(results, perms, rels):
    prelogits = np.zeros((E, H), dtype=np.float32)
    feat_n = np.zeros((N, CD), dtype=np.float32)
    for c in range(NCORES):
        out = results[c]
        pr = (
            np.asarray(out["prelog_r"])
            .transpose(0, 2, 1, 3)
            .reshape(NT, SLOTS, H)
        )
        mask = rels[c] >= 0.0
        prelogits[perms[c][mask]] = pr[mask] * SCALE
        fr = np.asarray(out["feat_r"]).reshape(NT * TILE, CD)
        feat_n[c * NPC : (c + 1) * NPC] = fr[:NPC]
    s = feat_n[:, CV:CD]
    unnorm = feat_n[:, :CV].reshape(N, H, 8, 3)
    feat = unnorm / s[:, :, None, None]
    feat = feat.reshape(N, 64, 3)
    out_0 = np.ascontiguousarray(feat[:, :32, :1])
    out_1 = np.ascontiguousarray(feat[:, 32:, :3])
    return out_0, out_1, prelogits


def _install_ntff_hook_shim():
    """The agent image's antenv lacks axon_hooks; recreate it so trace=True
    works under axon. Best-effort — degrades to no tracing on any failure."""
    import sys
    import types

    if "antenv.axon_hooks" in sys.modules:
        return
    try:
        from trn_agent_boot.trn_boot import _ntff_profile_via_ctypes

        hook = _ntff_profile_via_ctypes("/opt/axon/libaxon_pjrt.so")
        mod = types.ModuleType("antenv.axon_hooks")
        mod._hook = hook
        mod.set_axon_ntff_profile_hook = lambda h: setattr(mod, "_hook", h)
        mod.get_axon_ntff_profile_hook = lambda: mod._hook
        sys.modules["antenv.axon_hooks"] = mod
        import antenv

        antenv.axon_hooks = mod
    except Exception:
        pass


def kernel(value, key_feat, query_0, query_1, dst):
    global LAST_RESULTS
    from concourse import bass_utils

    nc = _get_compiled()
    in_maps, perms, rels = _host_prep(value, key_feat, query_0, query_1, dst)
    trace = os.environ.get("KERNEL_TRACE", "0") == "1"
    if trace:
        _install_ntff_hook_shim()
    res = bass_utils.run_bass_kernel_spmd(
        nc,
        in_maps,
        core_ids=list(range(NCORES)),
        trace=trace,
    )
    LAST_RESULTS = res
    return _host_unshard(res.results, perms, rels)


# revision 16
# speedup vs baseline: 1.5163x; 1.0584x over previous
"""AttentionSE3 message-passing kernel for 8 Trainium2 NeuronCores.

Strategy (node-sharded, zero device collectives):
  - The softmax over incoming edges of each dst node is computed WITHOUT the
    max-subtraction (prelogits are ~N(0, 0.35^2) so exp never overflows), and
    the division by the softmax denominator commutes with the segment-sum:
        feat[n] = (sum_{e->n} exp(pl_e) * v_e) / (sum_{e->n} exp(pl_e))
    so each core only needs the edges whose dst it owns.
  - Host: sort edges by dst, route each edge to the core owning its dst node
    (2500 nodes/core), group into 20 node-tiles of 128 nodes, pad each
    node-tile's edge list to ET*128 slots. Gather key/value/q[dst] rows into
    the padded layout.
  - Device (per core): for each node-tile, build a 0/1 one-hot matrix
    oh[e, n] = (rel[e] == n) on the VectorE, compute per-edge/per-head logits
    as an elementwise mul + grouped reduce, exp on ScalarE, then segment-sum
    s and exp*value with a single TensorE matmul  feat = oh^T @ [exv | ex].
  - Host: un-permute prelogits, divide feat by s, slice outputs.
"""

import math
import os

import numpy as np

# ---- problem constants (hardcoded; must match reference.setup_inputs) ----
N = 20000
E = 640000
H = 8
DHEAD = 16
F = 128            # NUM_FEATURES = H * DHEAD
CV = 192           # value channels flattened (64*3), layout [8h, 8c, 3d]
CD = CV + H        # 200: value channels + per-head ex column for s
NCORES = 8
NPC = N // NCORES  # 2500 nodes per core
TILE = 128
NT = 20            # node tiles per core  (20*128 = 2560 >= 2500)
ET = 34            # max edge tiles per node tile (max count 4252 <= 4352)
SLOTS = ET * TILE
ETC = 17           # edge tiles per chunk
NCH = ET // ETC    # 2
SCALE = 1.0 / math.sqrt(float(F))

_COMPILED = None
LAST_RESULTS = None


def _build_nc(nt_count=NT, et=ET, etc=ETC, compile=True, mode="fp16"):
    import concourse.bass as bass
    import concourse.tile as tile
    from concourse import bacc, mybir

    nch = et // etc
    assert nch * etc == et

    fp32 = mybir.dt.float32
    fp16 = mybir.dt.float16
    # dtype of the matmul operands (one-hot lhsT + exv rhs) and of the
    # key/qg/val transport
    if mode == "fp32r":
        fmm, fio = mybir.dt.float32r, fp32
        CDP = 256  # fp32r fast path needs matmul output free size >= 256
    elif mode == "fp16":
        fmm, fio = fp16, fp16
        CDP = CD
    else:
        fmm, fio = fp32, fp32
        CDP = CD
    nc = bacc.Bacc(
        "TRN2",
        target_bir_lowering=False,
        debug=False,
        enable_asserts=False,
        num_devices=NCORES,
    )

    key_d = nc.dram_tensor("key_r", [nt_count, TILE, et, F], fio, kind="ExternalInput").ap()
    qg_d = nc.dram_tensor("qg_r", [nt_count, TILE, et, F], fio, kind="ExternalInput").ap()
    val_d = nc.dram_tensor("val_r", [nt_count, TILE, et, CV], fio, kind="ExternalInput").ap()
    rel_d = nc.dram_tensor("rel_r", [nt_count, TILE, et], fp32, kind="ExternalInput").ap()
    feat_d = nc.dram_tensor("feat_r", [nt_count, TILE, CD], fp32, kind="ExternalOutput").ap()
    prelog_d = nc.dram_tensor(
        "prelog_r", [nt_count, TILE, et, H], fp32, kind="ExternalOutput"
    ).ap()

    with tile.TileContext(nc) as tc:
        with (
            tc.tile_pool(name="consts", bufs=1) as consts,
            tc.tile_pool(name="io", bufs=3) as io,
            tc.tile_pool(name="work", bufs=3) as work,
            tc.tile_pool(name="outp", bufs=2) as outp,
            tc.tile_pool(name="pfeat", bufs=2, space="PSUM") as pfeat,
        ):
            iota_t = consts.tile([TILE, TILE], fp32)
            nc.gpsimd.iota(
                iota_t[:],
                [[1, TILE]],
                channel_multiplier=0,
                allow_small_or_imprecise_dtypes=True,
            )

            for nt in range(nt_count):
                feat_p = pfeat.tile([TILE, CDP], fp32)
                for ch in range(nch):
                    et0 = ch * etc
                    k_t = io.tile([TILE, etc, F], fio, tag="k")
                    nc.sync.dma_start(k_t[:], key_d[nt, :, et0 : et0 + etc, :])
                    qg_t = io.tile([TILE, etc, F], fio, tag="qg")
                    nc.sync.dma_start(qg_t[:], qg_d[nt, :, et0 : et0 + etc, :])
                    v_t = io.tile([TILE, etc, CV], fio, tag="v")
                    nc.sync.dma_start(v_t[:], val_d[nt, :, et0 : et0 + etc, :])
                    rel_t = io.tile([TILE, etc], fp32, tag="rel")
                    nc.sync.dma_start(rel_t[:], rel_d[nt, :, et0 : et0 + etc])

                    # one-hot: oh[p, et, j] = (rel[p, et] == j)
                    oh_t = work.tile([TILE, etc, TILE], fmm, tag="oh")
                    nc.vector.tensor_tensor(
                        out=oh_t[:],
                        in0=iota_t[:].unsqueeze(1).to_broadcast([TILE, etc, TILE]),
                        in1=rel_t[:].unsqueeze(2).to_broadcast([TILE, etc, TILE]),
                        op=mybir.AluOpType.is_equal,
                    )

                    # kq = k * q_gathered; fp16 out keeps DVE in 2x mode
                    if mode == "fp16":
                        kq_t = work.tile([TILE, etc, F], fp16, tag="kq")
                    else:
                        kq_t = k_t
                    nc.vector.tensor_tensor(
                        out=kq_t[:], in0=k_t[:], in1=qg_t[:], op=mybir.AluOpType.mult
                    )
                    # logits[p, et, h] = sum_d kq[p, et, h, d]
                    logit_t = work.tile([TILE, etc, H], fp32, tag="logit")
                    if mode == "fp16":
                        # tree reduction via in-place halving adds (fp16 2x mode);
                        # final level converts to fp32
                        kq4 = kq_t[:].rearrange("p e (h d) -> p e h d", d=DHEAD)
                        for w in (8, 4, 2):
                            nc.vector.tensor_tensor(
                                out=kq4[:, :, :, 0:w],
                                in0=kq4[:, :, :, 0:w],
                                in1=kq4[:, :, :, w : 2 * w],
                                op=mybir.AluOpType.add,
                            )
                        nc.vector.tensor_tensor(
                            out=logit_t[:].rearrange("p e (h o) -> p e h o", o=1),
                            in0=kq4[:, :, :, 0:1],
                            in1=kq4[:, :, :, 1:2],
                            op=mybir.AluOpType.add,
                        )
                    else:
                        nc.vector.tensor_reduce(
                            out=logit_t[:],
                            in_=kq_t[:].rearrange("p e (h d) -> p e h d", d=DHEAD),
                            axis=mybir.AxisListType.X,
                            op=mybir.AluOpType.add,
                        )

                    # prelogits output = raw logits (host multiplies by SCALE)
                    nc.sync.dma_start(
                        prelog_d[nt, :, et0 : et0 + etc, :], logit_t[:]
                    )

                    # exv[:, :, 192:200] = exp(logits * SCALE)
                    exv_t = work.tile([TILE, etc, CDP], fmm, tag="exv")
                    nc.scalar.activation(
                        exv_t[:, :, CV:CD],
                        logit_t[:],
                        mybir.ActivationFunctionType.Exp,
                        scale=SCALE,
                    )
                    # exv[:, :, 0:192] = v * ex (broadcast ex over 24 channels)
                    # split across GpSimd/DVE to balance engine load
                    chunk_idx = nt * nch + ch
                    exv_eng = (
                        nc.vector
                        if (mode == "fp16" and chunk_idx % 4 == 1)
                        else nc.gpsimd
                    )
                    exv_eng.tensor_tensor(
                        out=exv_t[:, :, 0:CV].rearrange("p e (h c) -> p e h c", c=24),
                        in0=v_t[:].rearrange("p e (h c) -> p e h c", c=24),
                        in1=exv_t[:, :, CV:CD]
                        .unsqueeze(3)
                        .to_broadcast([TILE, etc, H, 24]),
                        op=mybir.AluOpType.mult,
                    )
                    # (fp32r pad columns CD:CDP are left uninitialized; garbage
                    # lands only in feat columns >= CD which the host ignores)

                    # scatter: feat[n, c] += sum_e oh[e, n] * exv[e, c]
                    for eti in range(etc):
                        nc.tensor.matmul(
                            feat_p[:],
                            oh_t[:, eti, :],
                            exv_t[:, eti, :],
                            start=(ch == 0 and eti == 0),
                            stop=(ch == nch - 1 and eti == etc - 1),
                        )

                feat_s = outp.tile([TILE, CD], fp32, tag="feat")
                nc.vector.tensor_copy(feat_s[:], feat_p[:, :CD])
                nc.sync.dma_start(feat_d[nt], feat_s[:])

    if compile:
        nc.compile()
    return nc


def _build_nc_v2(nt_count=NT, et=ET, etc=ETC, compile=True):
    """V2: q[dst] gathered ON DEVICE via PE (oh^T transpose + one-hot matmul
    against the node-tile's 128 queries) instead of a host-gathered qg input.
    Cuts input DMA by ~44.6MB/core (qg_r) at the cost of PE/ACT work."""
    import concourse.bass as bass
    import concourse.tile as tile
    from concourse import bacc, mybir
    from concourse.masks import make_identity

    nch = et // etc
    assert nch * etc == et
    fp32 = mybir.dt.float32
    nc = bacc.Bacc(
        "TRN2",
        target_bir_lowering=False,
        debug=False,
        enable_asserts=False,
        num_devices=NCORES,
    )

    key_d = nc.dram_tensor("key_r", [nt_count, TILE, et, F], fp32, kind="ExternalInput").ap()
    q_d = nc.dram_tensor("q_r", [nt_count, TILE, F], fp32, kind="ExternalInput").ap()
    val_d = nc.dram_tensor("val_r", [nt_count, TILE, et, CV], fp32, kind="ExternalInput").ap()
    rel_d = nc.dram_tensor("rel_r", [nt_count, TILE, et], fp32, kind="ExternalInput").ap()
    feat_d = nc.dram_tensor("feat_r", [nt_count, TILE, CD], fp32, kind="ExternalOutput").ap()
    prelog_d = nc.dram_tensor(
        "prelog_r", [nt_count, TILE, et, H], fp32, kind="ExternalOutput"
    ).ap()

    # eti groups of <=4 share one PSUM qg tile ([128, 512] = 1 bank)
    groups = []
    g0 = 0
    while g0 < etc:
        gs = min(4, etc - g0)
        groups.append((g0, gs))
        g0 += gs

    with tile.TileContext(nc) as tc:
        with (
            tc.tile_pool(name="consts", bufs=1) as consts,
            tc.tile_pool(name="io", bufs=3) as io,
            tc.tile_pool(name="qio", bufs=2) as qio,
            tc.tile_pool(name="work", bufs=3) as work,
            tc.tile_pool(name="oh2p", bufs=4) as oh2p,
            tc.tile_pool(name="outp", bufs=2) as outp,
            tc.tile_pool(name="pfeat", bufs=2, space="PSUM") as pfeat,
            tc.tile_pool(name="ptrans", bufs=2, space="PSUM") as ptrans,
            tc.tile_pool(name="pqg", bufs=2, space="PSUM") as pqg,
        ):
            iota_t = consts.tile([TILE, TILE], fp32)
            nc.gpsimd.iota(
                iota_t[:],
                [[1, TILE]],
                channel_multiplier=0,
                allow_small_or_imprecise_dtypes=True,
            )
            ident_t = consts.tile([TILE, TILE], fp32)
            make_identity(nc, ident_t[:])

            for nt in range(nt_count):
                feat_p = pfeat.tile([TILE, CD], fp32)
                q_t = qio.tile([TILE, F], fp32, tag="q")
                nc.sync.dma_start(q_t[:], q_d[nt])
                for ch in range(nch):
                    et0 = ch * etc
                    k_t = io.tile([TILE, etc, F], fp32, tag="k")
                    nc.sync.dma_start(k_t[:], key_d[nt, :, et0 : et0 + etc, :])
                    v_t = io.tile([TILE, etc, CV], fp32, tag="v")
                    nc.sync.dma_start(v_t[:], val_d[nt, :, et0 : et0 + etc, :])
                    rel_t = io.tile([TILE, etc], fp32, tag="rel")
                    nc.sync.dma_start(rel_t[:], rel_d[nt, :, et0 : et0 + etc])

                    oh_t = work.tile([TILE, etc, TILE], fp32, tag="oh")
                    nc.vector.tensor_tensor(
                        out=oh_t[:],
                        in0=iota_t[:].unsqueeze(1).to_broadcast([TILE, etc, TILE]),
                        in1=rel_t[:].unsqueeze(2).to_broadcast([TILE, etc, TILE]),
                        op=mybir.AluOpType.is_equal,
                    )

                    # on-device q gather: qg[e, f] = sum_n oh[e, n] * q[n, f]
                    for g0, gs in groups:
                        qg_p = pqg.tile([TILE, 4 * TILE], fp32, tag="qg")
                        for j in range(gs):
                            eti = g0 + j
                            ohT_p = ptrans.tile([TILE, TILE], fp32, tag="ohT")
                            nc.tensor.transpose(
                                ohT_p[:], oh_t[:, eti, :], ident_t[:]
                            )
                            oh2_s = oh2p.tile([TILE, TILE], fp32, tag="oh2")
                            nc.scalar.activation(
                                oh2_s[:],
                                ohT_p[:],
                                mybir.ActivationFunctionType.Copy,
                            )
                            nc.tensor.matmul(
                                qg_p[:, j * TILE : (j + 1) * TILE],
                                oh2_s[:],
                                q_t[:],
                                start=True,
                                stop=True,
                            )
                        # kq = k * qg (in place over k), one op per group
                        nc.vector.tensor_tensor(
                            out=k_t[:, g0 : g0 + gs, :],
                            in0=k_t[:, g0 : g0 + gs, :],
                            in1=qg_p[:, : gs * TILE].rearrange(
                                "p (e f) -> p e f", f=F
                            ),
                            op=mybir.AluOpType.mult,
                        )

                    logit_t = work.tile([TILE, etc, H], fp32, tag="logit")
                    nc.vector.tensor_reduce(
                        out=logit_t[:],
                        in_=k_t[:].rearrange("p e (h d) -> p e h d", d=DHEAD),
                        axis=mybir.AxisListType.X,
                        op=mybir.AluOpType.add,
                    )

                    nc.sync.dma_start(
                        prelog_d[nt, :, et0 : et0 + etc, :], logit_t[:]
                    )

                    exv_t = work.tile([TILE, etc, CD], fp32, tag="exv")
                    nc.scalar.activation(
                        exv_t[:, :, CV:CD],
                        logit_t[:],
                        mybir.ActivationFunctionType.Exp,
                        scale=SCALE,
                    )
                    nc.gpsimd.tensor_tensor(
                        out=exv_t[:, :, 0:CV].rearrange("p e (h c) -> p e h c", c=24),
                        in0=v_t[:].rearrange("p e (h c) -> p e h c", c=24),
                        in1=exv_t[:, :, CV:CD]
                        .unsqueeze(3)
                        .to_broadcast([TILE, etc, H, 24]),
                        op=mybir.AluOpType.mult,
                    )

                    for eti in range(etc):
                        nc.tensor.matmul(
                            feat_p[:],
                            oh_t[:, eti, :],
                            exv_t[:, eti, :],
                            start=(ch == 0 and eti == 0),
                            stop=(ch == nch - 1 and eti == etc - 1),
                        )

                feat_s = outp.tile([TILE, CD], fp32, tag="feat")
                nc.scalar.activation(
                    feat_s[:], feat_p[:], mybir.ActivationFunctionType.Copy
                )
                nc.sync.dma_start(feat_d[nt], feat_s[:])

    if compile:
        nc.compile()
    return nc


USE_V2 = os.environ.get("KERNEL_V2", "0") == "1"
MODE = os.environ.get("KERNEL_MODE", "fp16")
IO_DTYPE = {"fp16": np.float16}.get(MODE, np.float32)


def _get_compiled():
    global _COMPILED
    if _COMPILED is None:
        _COMPILED = _build_nc_v2() if USE_V2 else _build_nc(mode=MODE)
    return _COMPILED


def _host_prep(value, key_feat, query_0, query_1, dst):
    """Route edges to dst-owner cores, build padded per-core device inputs."""
    dst = np.asarray(dst).astype(np.int64)
    q_cat = np.concatenate(
        [np.asarray(query_0), np.asarray(query_1)], axis=-1
    ).reshape(N, F)
    key_feat = np.ascontiguousarray(np.asarray(key_feat, dtype=np.float32))
    val_flat = np.ascontiguousarray(
        np.asarray(value, dtype=np.float32).reshape(E, CV)
    )

    core_of = dst // NPC
    local = dst - core_of * NPC
    nt_of = local // TILE
    bucket = core_of * NT + nt_of
    order = np.argsort(bucket, kind="stable")
    counts = np.bincount(bucket, minlength=NCORES * NT).reshape(NCORES, NT)
    assert counts.max() <= SLOTS, f"padding overflow: {counts.max()} > {SLOTS}"

    starts = np.zeros(NCORES * NT + 1, dtype=np.int64)
    np.cumsum(counts.reshape(-1), out=starts[1:])

    in_maps = []
    perms = []
    rels = []
    for c in range(NCORES):
        perm = np.zeros((NT, SLOTS), dtype=np.int64)
        rel = np.full((NT, SLOTS), -1.0, dtype=np.float32)
        for t in range(NT):
            b = c * NT + t
            n_e = counts[c, t]
            sl = order[starts[b] : starts[b] + n_e]
            perm[t, :n_e] = sl
            rel[t, :n_e] = (local[sl] - t * TILE).astype(np.float32)
        perms.append(perm)
        rels.append(rel)

        key_r = (
            key_feat[perm]
            .reshape(NT, ET, TILE, F)
            .transpose(0, 2, 1, 3)
        )
        val_r = (
            val_flat[perm]
            .reshape(NT, ET, TILE, CV)
            .transpose(0, 2, 1, 3)
        )
        rel_r = rel.reshape(NT, ET, TILE).transpose(0, 2, 1)
        im = {
            "key_r": np.ascontiguousarray(key_r, dtype=IO_DTYPE),
            "val_r": np.ascontiguousarray(val_r, dtype=IO_DTYPE),
            "rel_r": np.ascontiguousarray(rel_r, dtype=np.float32),
        }
        if USE_V2:
            q_r = np.zeros((NT, TILE, F), dtype=np.float32)
            q_core = q_cat[c * NPC : (c + 1) * NPC]
            q_r.reshape(NT * TILE, F)[:NPC] = q_core
            im["q_r"] = q_r
        else:
            qg_r = (
                q_cat[dst[perm]]
                .reshape(NT, ET, TILE, F)
                .transpose(0, 2, 1, 3)
            )
            im["qg_r"] = np.ascontiguousarray(qg_r, dtype=IO_DTYPE)
        in_maps.append(im)
    return in_maps, perms, rels


def _host_unshard(results, perms, rels):
    prelogits = np.zeros((E, H), dtype=np.float32)
    feat_n = np.zeros((N, CD), dtype=np.float32)
    for c in range(NCORES):
        out = results[c]
        pr = (
            np.asarray(out["prelog_r"])
            .transpose(0, 2, 1, 3)
            .reshape(NT, SLOTS, H)
        )
        mask = rels[c] >= 0.0
        prelogits[perms[c][mask]] = pr[mask] * SCALE
        fr = np.asarray(out["feat_r"]).reshape(NT * TILE, CD)
        feat_n[c * NPC : (c + 1) * NPC] = fr[:NPC]
    s = feat_n[:, CV:CD]
    unnorm = feat_n[:, :CV].reshape(N, H, 8, 3)
    feat = unnorm / s[:, :, None, None]
    feat = feat.reshape(N, 64, 3)
    out_0 = np.ascontiguousarray(feat[:, :32, :1])
    out_1 = np.ascontiguousarray(feat[:, 32:, :3])
    return out_0, out_1, prelogits


def _install_ntff_hook_shim():
    """The agent image's antenv lacks axon_hooks; recreate it so trace=True
    works under axon. Best-effort — degrades to no tracing on any failure."""
    import sys
    import types

    if "antenv.axon_hooks" in sys.modules:
        return
    try:
        from trn_agent_boot.trn_boot import _ntff_profile_via_ctypes

        hook = _ntff_profile_via_ctypes("/opt/axon/libaxon_pjrt.so")
        mod = types.ModuleType("antenv.axon_hooks")
        mod._hook = hook
        mod.set_axon_ntff_profile_hook = lambda h: setattr(mod, "_hook", h)
        mod.get_axon_ntff_profile_hook = lambda: mod._hook
        sys.modules["antenv.axon_hooks"] = mod
        import antenv

        antenv.axon_hooks = mod
    except Exception:
        pass


def kernel(value, key_feat, query_0, query_1, dst):
    global LAST_RESULTS
    from concourse import bass_utils

    nc = _get_compiled()
    in_maps, perms, rels = _host_prep(value, key_feat, query_0, query_1, dst)
    trace = os.environ.get("KERNEL_TRACE", "0") == "1"
    if trace:
        _install_ntff_hook_shim()
    res = bass_utils.run_bass_kernel_spmd(
        nc,
        in_maps,
        core_ids=list(range(NCORES)),
        trace=trace,
    )
    LAST_RESULTS = res
    return _host_unshard(res.results, perms, rels)
